# revision 28
# baseline (speedup 1.0000x reference)
"""Trainium2 Bass kernel for nn_BaseBranch_6811818132502 (dense_cnn).

Strategy:
 - Host-side (exact, verified vs reference): fold the channel-permutation
   einsum into conv1 weights, fold the rot90/rot-back pairs into spatially
   rotated 3x3 kernels, fold BN scale into the conv weights and BN
   shift+conv-bias into per-channel biases, and replace the pad-20 odd
   passes with pad-4 (receptive field is 5).  The module becomes 8 passes
   of [conv3x3(d=1) -> conv3x3(d=2) -> conv3x3(d=2)], each with fused
   bias+relu, then a global max over 8*64 channels, sigmoid, clip.
 - Device-side: data-parallel over batch (1 image per core, 8 cores).
   bf16 weights/activations (fp32 PSUM accumulation); same-parity anchor
   passes run in PAIRS to fill the 128-wide PE array:
     * conv1: both passes share the rhs ([x | x shifted 1 row] for
       kh-pairing), one dense K=128 M=128 matmul per tap-group -- 6
       matmuls per chunk compute BOTH passes' conv1.
     * conv2/conv3: per-pass buffers hold [plain | +2-row shift] (pass A)
       and [+2-row shift | plain] (pass B, flipped so its PSUM eviction
       stays partition-aligned).  Each tap-column group is one K=128 M=64
       matmul per pass; the two passes run as concurrent column tiles
       (tile positions (0,0) / (0,64)) -- 6 matmul slots per chunk for
       both passes.
   Evictions are split across engines: ScalarE does pass-A bias+relu,
   DVE does pass-B (tensor_scalar add-bias/max-0), GpSimd+DVE maintain
   the shifted halves with in-SBUF copies.  conv3 evicts bias-only and
   DVE keeps a pair-packed running channel max; a PE-transpose + DVE-max
   tree reduces over all 128 packed channels at the end.
"""
import sys
import os
import math

for _p in ("/opt/trn_rl_repo", "/root/.axon_site/_ro/trn_rl_repo"):
    if os.path.isdir(_p) and _p not in sys.path:
        sys.path.insert(0, _p)

import numpy as np
import ml_dtypes

import concourse.bass as bass
import concourse.mybir as mybir
import concourse.tile as tile
from concourse import bacc, masks
from concourse.bass_utils import run_bass_kernel_spmd
from contextlib import ExitStack

F32 = mybir.dt.float32
BF16 = mybir.dt.bfloat16
NP_BF16 = np.dtype(ml_dtypes.bfloat16)

BN_EPS = 1e-5
C = 64            # channels
H = W = 96        # map size
B = 8             # batch == n_cores
PAD = 4           # explicit pad for odd passes (exact; receptive field 5)

XO_S = H + 2 * PAD + 2      # 106: x + pad4 + conv1 halo 1
Y1_S = H + 2 * PAD          # 104: conv1 odd output domain (even uses interior)
Y2_S = H + 4                # 100: conv2 output domain (coords -2..97)

# geometry per (layer, parity): dilation, rhs row/col base offset,
# out rows/cols, output write offset, rows per PSUM chunk (n = rch*ow <= 512)
GEOM = {
    (0, 0): dict(d=1, off=4, oh=96,  ow=96,  woff=4, rch=5),   # conv1 even
    (0, 1): dict(d=1, off=0, oh=104, ow=104, woff=0, rch=4),   # conv1 odd
    (1, 0): dict(d=2, off=2, oh=96,  ow=96,  woff=2, rch=5),   # conv2 even
    (1, 1): dict(d=2, off=0, oh=100, ow=100, woff=0, rch=5),   # conv2 odd
    (2, 0): dict(d=2, off=0, oh=96,  ow=96,  woff=None, rch=5),  # conv3 -> ACC
    (2, 1): dict(d=2, off=0, oh=96,  ow=96,  woff=None, rch=5),
}
# evens first: odd passes overwrite the zero borders of y1/y2 that even
# passes rely on for implicit padding
PAIR_SEQ = [(0, 2), (4, 6), (1, 3), (5, 7)]
C1_COL = 0               # conv1 tap j at j*128 (A cols +0:64, B cols +64:128)
C2_COL = 6 * 128         # conv2 group j, half h at C2_COL + (2j+h)*64
C3_COL = C2_COL + 12 * 64
W_BLOB_COLS = C3_COL + 12 * 64   # 2304

_PROGRAM_CACHE = {}
TRACE = False
LAST_EXEC_NS = None


def _build_program():
    nc = bacc.Bacc("TRN2", target_bir_lowering=False, debug=False, num_devices=B)
    # x2_in is pre-padded and pair-packed on the host: partitions 0:64 hold
    # x with its zero ring, 64:128 the same shifted up one row (conv1
    # kh-pairing).  Contiguous per partition -> large DMA packets.
    x2_in = nc.dram_tensor("x2_in", [128, XO_S, XO_S], BF16, kind="ExternalInput")
    w_in = nc.dram_tensor("w_in", [4, 128, W_BLOB_COLS], BF16, kind="ExternalInput")
    bi_in = nc.dram_tensor("bi_in", [128, 3], F32, kind="ExternalInput")
    o_out = nc.dram_tensor("o_out", [1, H * W], F32, kind="ExternalOutput")

    with ExitStack() as ctx:
        tc = ctx.enter_context(tile.TileContext(nc))
        bigpool = ctx.enter_context(tc.tile_pool(name="big", bufs=1))
        wpool = ctx.enter_context(tc.tile_pool(name="wts", bufs=2))
        evpool = ctx.enter_context(tc.tile_pool(name="ev", bufs=4))
        psum = ctx.enter_context(tc.tile_pool(name="ps", bufs=6, space="PSUM"))
        tpsum = ctx.enter_context(tc.tile_pool(name="tps", bufs=2, space="PSUM"))

        xo = bigpool.tile([128, XO_S, XO_S], BF16)
        y1a = bigpool.tile([128, Y1_S, Y1_S], BF16)
        y1b = bigpool.tile([128, Y1_S, Y1_S], BF16)
        y2a = bigpool.tile([128, Y2_S, Y2_S], BF16)
        y2b = bigpool.tile([128, Y2_S, Y2_S], BF16)
        # column-shifted variants of y2 for the conv3 kh2-row kw-pairing:
        # y2c_a = [plain | plain<<2cols], y2c_b flipped
        y2c_a = bigpool.tile([128, Y2_S, Y2_S], BF16)
        y2c_b = bigpool.tile([128, Y2_S, Y2_S], BF16)
        acc = bigpool.tile([128, H * W], F32)
        bit = bigpool.tile([128, 3], F32)

        # pair-0 conv1 weights go out first, split across two queues --
        # they gate matmul #1
        wt0 = wpool.tile([128, W_BLOB_COLS], BF16, tag="wt")
        nc.sync.dma_start(out=wt0[:, 0:C2_COL // 2], in_=w_in[0, :, 0:C2_COL // 2])
        nc.scalar.dma_start(out=wt0[:, C2_COL // 2:C2_COL],
                            in_=w_in[0, :, C2_COL // 2:C2_COL])
        # xo: one pre-built DRAM image, DMAed in row segments spread across
        # the three DMA-capable queues so conv1 can start as soon as the
        # first rows land (a row is complete only when its segment's DMA
        # finishes, so the first segment is small).
        XSEG = [0, 12, 26, 42, 58, 74, 90, XO_S]
        seg_q = [nc.gpsimd, nc.sync, nc.scalar, nc.gpsimd,
                 nc.sync, nc.scalar, nc.gpsimd]
        for si in range(7):
            r0, r1 = XSEG[si], XSEG[si + 1]
            seg_q[si].dma_start(out=xo[:, r0:r1, :], in_=x2_in[:, r0:r1, :])
        nc.gpsimd.dma_start(out=bit, in_=bi_in[:, :])
        # preload the sigmoid ACT table set now (relu/identity are filler in
        # every set) so the tail doesn't pay a ~2.7us table switch
        dum = bigpool.tile([1, 8], F32)
        nc.vector.memset(dum, 0.0)
        nc.scalar.activation(out=dum, in_=dum,
                             func=mybir.ActivationFunctionType.Sigmoid)
        # warm up the PE HAM clock gate (~3.4us of activity -> 2.4 GHz)
        # while the first DMAs land
        wrm = bigpool.tile([128, 512], BF16)
        nc.vector.memset(wrm, 0.25)
        for _ in range(10):
            ptw = psum.tile([128, 512], F32, tag="pt")
            nc.tensor.matmul(ptw[:, 0:448], wrm[:, 0:128], wrm[:, 0:448],
                             start=True, stop=True)
        for buf, S in ((y1a, Y1_S), (y1b, Y1_S), (y2a, Y2_S), (y2b, Y2_S),
                       (y2c_a, Y2_S), (y2c_b, Y2_S)):
            nc.gpsimd.memset(buf[:, 0:4, :], 0.0)
            nc.gpsimd.memset(buf[:, S - 8:S, :], 0.0)
            nc.gpsimd.memset(buf[:, 4:S - 8, 0:4], 0.0)
            nc.gpsimd.memset(buf[:, 4:S - 8, S - 8:S], 0.0)
        nc.gpsimd.memset(acc, 0.0)

        bufsA = [xo, y1a, y2a, None]
        bufsB = [xo, y1b, y2b, None]
        SH = 2  # shifted-half row offset of y buffers (= conv2/3 dilation)

        for pi, (pa, pb) in enumerate(PAIR_SEQ):
            parity = pa % 2
            if pi == 0:
                wt = wt0
            else:
                wt = wpool.tile([128, W_BLOB_COLS], BF16, tag="wt")
                # conv1 columns first -- they gate the pair's first matmuls
                nc.sync.dma_start(out=wt[:, 0:C2_COL], in_=w_in[pi, :, 0:C2_COL])
            nc.sync.dma_start(out=wt[:, C2_COL:W_BLOB_COLS],
                              in_=w_in[pi, :, C2_COL:W_BLOB_COLS])

            for l in range(3):
                g = GEOM[(l, parity)]
                d, off, oh, ow, woff, rch = (g["d"], g["off"], g["oh"], g["ow"],
                                             g["woff"], g["rch"])
                h0 = 0
                pend_a = None   # start row of a pending shift-DMA group
                while h0 < oh:
                    rr = min(rch, oh - h0)
                    n = rr * ow
                    pt = psum.tile([128, 512], F32, tag="pt")
                    if l == 0:
                        # conv1: both passes in one dense K=128 matmul per tap
                        for j in range(6):
                            kw = j % 3
                            kh0 = 0 if j < 3 else 2
                            rbase = h0 + kh0 * d + off
                            cbase = kw * d + off
                            rhs = xo[0:128, rbase:rbase + rr, cbase:cbase + ow]
                            nc.tensor.matmul(pt[:, 0:n],
                                             wt[:, j * 128:(j + 1) * 128], rhs,
                                             start=(j == 0), stop=(j == 5))
                    elif l == 1:
                        # conv2: per-pass K=128 kh-packed matmuls as two
                        # concurrent column tiles (0,0)/(0,64)
                        for j in range(6):
                            kw = j % 3
                            kh0 = 0 if j < 3 else 2
                            rbase = h0 + kh0 * d + off
                            cbase = kw * d + off
                            rhsA = y1a[0:128, rbase:rbase + rr, cbase:cbase + ow]
                            rhsB = y1b[0:128, rbase:rbase + rr, cbase:cbase + ow]
                            cA = C2_COL + (2 * j) * 64
                            cB = C2_COL + (2 * j + 1) * 64
                            nc.tensor.matmul(pt[0:64, 0:n],
                                             wt[:, cA:cA + 64], rhsA,
                                             start=(j == 0), stop=(j == 5))
                            nc.tensor.matmul(pt[64:128, 0:n],
                                             wt[:, cB:cB + 64], rhsB,
                                             start=(j == 0), stop=(j == 5))
                    else:
                        # conv3: 5 slots -- 3 kh01-kw groups from the row
                        # buffers, the kh2 (kw0,kw1) pair from the column-
                        # shifted buffers, and kh2-kw2 half-width
                        for g in range(5):
                            if g < 3:
                                rbase, cbase = h0, g * d
                                sA, sB = y2a, y2b
                            elif g == 3:
                                rbase, cbase = h0 + 2 * d, 0
                                sA, sB = y2c_a, y2c_b
                            else:
                                rbase, cbase = h0 + 2 * d, 2 * d
                                sA, sB = y2a, y2b
                            rhsA = sA[0:128, rbase:rbase + rr, cbase:cbase + ow]
                            rhsB = sB[0:128, rbase:rbase + rr, cbase:cbase + ow]
                            cA = C3_COL + (2 * g) * 64
                            cB = C3_COL + (2 * g + 1) * 64
                            nc.tensor.matmul(pt[0:64, 0:n],
                                             wt[:, cA:cA + 64], rhsA,
                                             start=(g == 0), stop=(g == 4))
                            nc.tensor.matmul(pt[64:128, 0:n],
                                             wt[:, cB:cB + 64], rhsB,
                                             start=(g == 0), stop=(g == 4))
                    if l < 2:
                        dstA, dstB = bufsA[l + 1], bufsB[l + 1]
                        a = h0 + woff
                        # pass A plain half (partitions 0:64): ScalarE
                        nc.scalar.activation(
                            out=dstA[0:64, a:a + rr, woff:woff + ow],
                            in_=pt[0:64, 0:n].rearrange("p (r c) -> p r c", r=rr),
                            func=mybir.ActivationFunctionType.Relu,
                            bias=bit[0:64, l:l + 1])
                        # pass B plain half (partitions 64:128): DVE
                        nc.vector.tensor_scalar(
                            dstB[64:128, a:a + rr, woff:woff + ow],
                            pt[64:128, 0:n].rearrange("p (r c) -> p r c", r=rr),
                            bit[64:128, l:l + 1], 0.0,
                            mybir.AluOpType.add, mybir.AluOpType.max)
                        # shifted halves: rows [u0, u1) <- plain rows +SH,
                        # maintained with SBUF->SBUF DMAs (2 chunks per DMA)
                        # issued from the otherwise-idle sync/gpsimd queues
                        if pend_a is None:
                            pend_a = a
                        if h0 + rr >= oh or a + rr - pend_a >= 2 * rch:
                            u0, u1 = max(0, pend_a - SH), a + rr - SH
                            if u1 > u0:
                                nc.sync.dma_start(
                                    out=dstA[64:128, u0:u1, :],
                                    in_=dstA[0:64, u0 + SH:u1 + SH, :])
                                nc.gpsimd.dma_start(
                                    out=dstB[0:64, u0:u1, :],
                                    in_=dstB[64:128, u0 + SH:u1 + SH, :])
                            if l == 1:
                                # feed the conv3 column-shifted buffers
                                r0, r1 = pend_a, a + rr
                                nc.scalar.dma_start(
                                    out=y2c_a[0:64, r0:r1, :],
                                    in_=y2a[0:64, r0:r1, :])
                                nc.scalar.dma_start(
                                    out=y2c_a[64:128, r0:r1, 0:Y2_S - SH],
                                    in_=y2a[0:64, r0:r1, SH:Y2_S])
                                nc.gpsimd.dma_start(
                                    out=y2c_b[64:128, r0:r1, :],
                                    in_=y2b[64:128, r0:r1, :])
                                nc.gpsimd.dma_start(
                                    out=y2c_b[0:64, r0:r1, 0:Y2_S - SH],
                                    in_=y2b[64:128, r0:r1, SH:Y2_S])
                            pend_a = None
                    else:
                        # conv3: bias then running channel max; acc is
                        # pair-packed [128, H*W] -- cross-half max happens in
                        # the final channel reduction
                        tmp = evpool.tile([128, 512], F32, tag="ev")
                        nc.scalar.activation(
                            out=tmp[:, 0:n], in_=pt[:, 0:n],
                            func=mybir.ActivationFunctionType.Identity,
                            bias=bit[:, 2:3])
                        nc.vector.tensor_max(
                            acc[:, h0 * W:h0 * W + n],
                            acc[:, h0 * W:h0 * W + n],
                            tmp[:, 0:n])
                    h0 += rr

        # channel-max reduction: PE-transpose 128-col blocks of acc into PSUM
        # ([128, 128] -> [128, 128]) and reduce over the free dim (both pair
        # halves' channels) on DVE.  72 blocks in 18 groups of 4; each
        # group's transposes start as soon as the conv3 maxes land.
        ident = bigpool.tile([128, 128], F32)
        masks.make_identity(nc, ident)
        red = bigpool.tile([128, 72], F32)
        NB = (H * W) // 128  # 72 blocks
        for g in range(NB // 4):
            ps = tpsum.tile([128, 512], F32, tag="tp")
            for b in range(4):
                blk = g * 4 + b
                nc.tensor.transpose(ps[:, b * 128:(b + 1) * 128],
                                    acc[:, blk * 128:(blk + 1) * 128],
                                    ident[:, :])
            nc.vector.tensor_reduce(out=red[:, g * 4:(g + 1) * 4],
                                    in_=ps.rearrange("p (b c) -> p b c", b=4),
                                    axis=mybir.AxisListType.X,
                                    op=mybir.AluOpType.max)
        ps2 = tpsum.tile([128, 512], F32, tag="tp")
        nc.tensor.transpose(ps2[0:72, 0:128], red[:, :], ident[:, :])
        rsb = bigpool.tile([72, 128], F32)
        nc.scalar.activation(out=rsb, in_=ps2[0:72, 0:128],
                             func=mybir.ActivationFunctionType.Sigmoid)
        nc.vector.tensor_scalar(rsb, rsb, 1e-4, 1.0 - 1e-4,
                                mybir.AluOpType.max, mybir.AluOpType.min)
        nc.sync.dma_start(
            out=o_out.ap().rearrange("a (c r) -> a c r", r=128), in_=rsb)
    nc.compile()
    return nc


def _fold_weights(perms, dcn_w, dcn_b, conv2_w, conv2_b, conv3_w, conv3_b,
                  bn_gamma, bn_beta, bn_mean, bn_var):
    """Fold rotations/permutation/BN on the host. Returns (w_blob, biases).

    BN scale is folded into the conv weights (per OUT channel); biases keep
    the BN shift + scaled conv bias.
    """
    biases = np.empty((128, 3), np.float32)
    scs = []
    conv_bs = [dcn_b, conv2_b, conv3_b]
    for l in range(3):
        s = bn_gamma[l] / np.sqrt(bn_var[l] + BN_EPS)
        scs.append(s)
        bl = bn_beta[l] - bn_mean[l] * s + conv_bs[l] * s
        biases[0:C, l] = bl
        biases[C:128, l] = bl

    w_blob = np.zeros((4, 128, W_BLOB_COLS), np.float32)
    base_ws = [dcn_w, conv2_w, conv3_w]
    for pi, pair in enumerate(PAIR_SEQ):
        for half, p in enumerate(pair):
            k = p % 4
            # conv1: rotation + channel permutation + BN scale folded
            w1 = np.rot90(base_ws[0], k=-k, axes=(-2, -1))
            w1 = np.einsum('omhw,mj->ojhw', w1, perms[p], optimize=True)
            w1 = w1 * scs[0][:, None, None, None]
            for j in range(6):
                kw = j % 3
                col = j * 128 + half * 64
                if j < 3:
                    w_blob[pi, 0:C, col:col + C] = w1[:, :, 0, kw].T
                    w_blob[pi, C:128, col:col + C] = w1[:, :, 1, kw].T
                else:
                    w_blob[pi, 0:C, col:col + C] = w1[:, :, 2, kw].T
            # conv2: kh-packed K=128 per-pass tiles.  Pass A buffers are
            # [plain | +2shift] (plain rows 0:64 -> kh0/kh2 taps on K rows
            # 0:64); pass B buffers are flipped [+2shift | plain] (plain on
            # K rows 64:128).
            w2 = np.rot90(base_ws[1], k=-k, axes=(-2, -1))
            w2 = w2 * scs[1][:, None, None, None]
            for j in range(6):
                kw = j % 3
                col = C2_COL + (2 * j + half) * 64
                if half == 0:
                    if j < 3:
                        w_blob[pi, 0:C, col:col + C] = w2[:, :, 0, kw].T
                        w_blob[pi, C:128, col:col + C] = w2[:, :, 1, kw].T
                    else:
                        w_blob[pi, 0:C, col:col + C] = w2[:, :, 2, kw].T
                else:
                    if j < 3:
                        w_blob[pi, C:128, col:col + C] = w2[:, :, 0, kw].T
                        w_blob[pi, 0:C, col:col + C] = w2[:, :, 1, kw].T
                    else:
                        w_blob[pi, C:128, col:col + C] = w2[:, :, 2, kw].T
            # conv3: 5 groups -- g<3: (kh0,kh1) x kw via row buffers;
            # g==3: (kh2,kw0)+(kh2,kw1) via column-shifted buffers;
            # g==4: (kh2,kw2) half-width
            w3 = np.rot90(base_ws[2], k=-k, axes=(-2, -1))
            w3 = w3 * scs[2][:, None, None, None]
            for g in range(5):
                col = C3_COL + (2 * g + half) * 64
                if g < 3:
                    t0, t1 = (0, g), (1, g)
                elif g == 3:
                    t0, t1 = (2, 0), (2, 1)
                else:
                    t0, t1 = (2, 2), None
                if half == 0:   # plain on rows 0:64, shifted on 64:128
                    w_blob[pi, 0:C, col:col + C] = w3[:, :, t0[0], t0[1]].T
                    if t1 is not None:
                        w_blob[pi, C:128, col:col + C] = w3[:, :, t1[0], t1[1]].T
                else:           # flipped
                    w_blob[pi, C:128, col:col + C] = w3[:, :, t0[0], t0[1]].T
                    if t1 is not None:
                        w_blob[pi, 0:C, col:col + C] = w3[:, :, t1[0], t1[1]].T
    return w_blob, biases


def kernel(x, perms, dcn_w, dcn_b, conv2_w, conv2_b, conv3_w, conv3_b,
           bn_gamma, bn_beta, bn_mean, bn_var):
    global LAST_EXEC_NS
    x = np.ascontiguousarray(np.asarray(x, np.float32))
    args = [np.asarray(a, np.float32) for a in
            (perms, dcn_w, dcn_b, conv2_w, conv2_b, conv3_w, conv3_b,
             bn_gamma, bn_beta, bn_mean, bn_var)]
    w_blob, biases = _fold_weights(*args)
    w_blob = w_blob.astype(NP_BF16)
    x_bf = x.astype(NP_BF16)
    # pre-padded pair-packed image: [x + zero ring | same shifted up 1 row]
    P1 = PAD + 1
    x2 = np.zeros((B, 128, XO_S, XO_S), dtype=NP_BF16)
    x2[:, 0:C, P1:P1 + H, P1:P1 + W] = x_bf
    x2[:, C:128, P1 - 1:P1 - 1 + H, P1:P1 + W] = x_bf

    if "prog" not in _PROGRAM_CACHE:
        _PROGRAM_CACHE["prog"] = _build_program()
    nc = _PROGRAM_CACHE["prog"]

    in_maps = [{
        "x2_in": np.ascontiguousarray(x2[b]),
        "w_in": w_blob,
        "bi_in": biases,
    } for b in range(B)]

    r = run_bass_kernel_spmd(nc, in_maps, core_ids=list(range(B)), trace=TRACE)
    LAST_EXEC_NS = r.exec_time_ns
    out = np.stack([r.results[b]["o_out"].reshape(1, H, W) for b in range(B)])
    return out.astype(np.float32)


# revision 29
# speedup vs baseline: 1.0035x; 1.0035x over previous
"""Trainium2 Bass kernel for nn_BaseBranch_6811818132502 (dense_cnn).

Strategy:
 - Host-side (exact, verified vs reference): fold the channel-permutation
   einsum into conv1 weights, fold the rot90/rot-back pairs into spatially
   rotated 3x3 kernels, fold BN scale into the conv weights and BN
   shift+conv-bias into per-channel biases, and replace the pad-20 odd
   passes with pad-4 (receptive field is 5).  The module becomes 8 passes
   of [conv3x3(d=1) -> conv3x3(d=2) -> conv3x3(d=2)], each with fused
   bias+relu, then a global max over 8*64 channels, sigmoid, clip.
 - Device-side: data-parallel over batch (1 image per core, 8 cores).
   bf16 weights/activations (fp32 PSUM accumulation); same-parity anchor
   passes run in PAIRS to fill the 128-wide PE array:
     * conv1: both passes share the rhs ([x | x shifted 1 row] for
       kh-pairing), one dense K=128 M=128 matmul per tap-group -- 6
       matmuls per chunk compute BOTH passes' conv1.
     * conv2/conv3: per-pass buffers hold [plain | +2-row shift] (pass A)
       and [+2-row shift | plain] (pass B, flipped so its PSUM eviction
       stays partition-aligned).  Each tap-column group is one K=128 M=64
       matmul per pass; the two passes run as concurrent column tiles
       (tile positions (0,0) / (0,64)) -- 6 matmul slots per chunk for
       both passes.
   Evictions are split across engines: ScalarE does pass-A bias+relu,
   DVE does pass-B (tensor_scalar add-bias/max-0), GpSimd+DVE maintain
   the shifted halves with in-SBUF copies.  conv3 evicts bias-only and
   DVE keeps a pair-packed running channel max; a PE-transpose + DVE-max
   tree reduces over all 128 packed channels at the end.
"""
import sys
import os
import math

for _p in ("/opt/trn_rl_repo", "/root/.axon_site/_ro/trn_rl_repo"):
    if os.path.isdir(_p) and _p not in sys.path:
        sys.path.insert(0, _p)

import numpy as np
import ml_dtypes

import concourse.bass as bass
import concourse.mybir as mybir
import concourse.tile as tile
from concourse import bacc, masks
from concourse.bass_utils import run_bass_kernel_spmd
from contextlib import ExitStack

F32 = mybir.dt.float32
BF16 = mybir.dt.bfloat16
NP_BF16 = np.dtype(ml_dtypes.bfloat16)

BN_EPS = 1e-5
C = 64            # channels
H = W = 96        # map size
B = 8             # batch == n_cores
PAD = 4           # explicit pad for odd passes (exact; receptive field 5)

XO_S = H + 2 * PAD + 2      # 106: x + pad4 + conv1 halo 1
Y1_S = H + 2 * PAD          # 104: conv1 odd output domain (even uses interior)
Y2_S = H + 4                # 100: conv2 output domain (coords -2..97)

# geometry per (layer, parity): dilation, rhs row/col base offset,
# out rows/cols, output write offset, rows per PSUM chunk (n = rch*ow <= 512)
GEOM = {
    (0, 0): dict(d=1, off=4, oh=96,  ow=96,  woff=4, rch=5),   # conv1 even
    (0, 1): dict(d=1, off=0, oh=104, ow=104, woff=0, rch=4),   # conv1 odd
    (1, 0): dict(d=2, off=2, oh=96,  ow=96,  woff=2, rch=5),   # conv2 even
    (1, 1): dict(d=2, off=0, oh=100, ow=100, woff=0, rch=5),   # conv2 odd
    (2, 0): dict(d=2, off=0, oh=96,  ow=96,  woff=None, rch=5),  # conv3 -> ACC
    (2, 1): dict(d=2, off=0, oh=96,  ow=96,  woff=None, rch=5),
}
# evens first: odd passes overwrite the zero borders of y1/y2 that even
# passes rely on for implicit padding
PAIR_SEQ = [(0, 2), (4, 6), (1, 3), (5, 7)]
C1_COL = 0               # conv1 tap j at j*128 (A cols +0:64, B cols +64:128)
C2_COL = 6 * 128         # conv2 group j, half h at C2_COL + (2j+h)*64
C3_COL = C2_COL + 12 * 64
W_BLOB_COLS = C3_COL + 12 * 64   # 2304

_PROGRAM_CACHE = {}
TRACE = False
LAST_EXEC_NS = None


def _build_program():
    nc = bacc.Bacc("TRN2", target_bir_lowering=False, debug=False, num_devices=B)
    # x2_in is pre-padded and pair-packed on the host: partitions 0:64 hold
    # x with its zero ring, 64:128 the same shifted up one row (conv1
    # kh-pairing).  Contiguous per partition -> large DMA packets.
    x2_in = nc.dram_tensor("x2_in", [128, XO_S, XO_S], BF16, kind="ExternalInput")
    w_in = nc.dram_tensor("w_in", [4, 128, W_BLOB_COLS], BF16, kind="ExternalInput")
    bi_in = nc.dram_tensor("bi_in", [128, 3], F32, kind="ExternalInput")
    o_out = nc.dram_tensor("o_out", [1, H * W], F32, kind="ExternalOutput")

    with ExitStack() as ctx:
        tc = ctx.enter_context(tile.TileContext(nc))
        bigpool = ctx.enter_context(tc.tile_pool(name="big", bufs=1))
        wpool = ctx.enter_context(tc.tile_pool(name="wts", bufs=2))
        evpool = ctx.enter_context(tc.tile_pool(name="ev", bufs=4))
        psum = ctx.enter_context(tc.tile_pool(name="ps", bufs=6, space="PSUM"))
        tpsum = ctx.enter_context(tc.tile_pool(name="tps", bufs=2, space="PSUM"))

        xo = bigpool.tile([128, XO_S, XO_S], BF16)
        y1a = bigpool.tile([128, Y1_S, Y1_S], BF16)
        y1b = bigpool.tile([128, Y1_S, Y1_S], BF16)
        y2a = bigpool.tile([128, Y2_S, Y2_S], BF16)
        y2b = bigpool.tile([128, Y2_S, Y2_S], BF16)
        acc = bigpool.tile([128, H * W], F32)
        bit = bigpool.tile([128, 3], F32)

        # pair-0 conv1 weights go out first, split across two queues --
        # they gate matmul #1
        wt0 = wpool.tile([128, W_BLOB_COLS], BF16, tag="wt")
        nc.sync.dma_start(out=wt0[:, 0:C2_COL // 2], in_=w_in[0, :, 0:C2_COL // 2])
        nc.scalar.dma_start(out=wt0[:, C2_COL // 2:C2_COL],
                            in_=w_in[0, :, C2_COL // 2:C2_COL])
        # xo: one pre-built DRAM image, DMAed in row segments spread across
        # the three DMA-capable queues so conv1 can start as soon as the
        # first rows land (a row is complete only when its segment's DMA
        # finishes, so the first segment is small).
        XSEG = [0, 12, 26, 42, 58, 74, 90, XO_S]
        seg_q = [nc.gpsimd, nc.sync, nc.scalar, nc.gpsimd,
                 nc.sync, nc.scalar, nc.gpsimd]
        for si in range(7):
            r0, r1 = XSEG[si], XSEG[si + 1]
            seg_q[si].dma_start(out=xo[:, r0:r1, :], in_=x2_in[:, r0:r1, :])
        nc.gpsimd.dma_start(out=bit, in_=bi_in[:, :])
        # preload the sigmoid ACT table set now (relu/identity are filler in
        # every set) so the tail doesn't pay a ~2.7us table switch
        dum = bigpool.tile([1, 8], F32)
        nc.vector.memset(dum, 0.0)
        nc.scalar.activation(out=dum, in_=dum,
                             func=mybir.ActivationFunctionType.Sigmoid)
        # warm up the PE HAM clock gate (~3.4us of activity -> 2.4 GHz)
        # while the first DMAs land
        wrm = bigpool.tile([128, 512], BF16)
        nc.vector.memset(wrm, 0.25)
        for _ in range(10):
            ptw = psum.tile([128, 512], F32, tag="pt")
            nc.tensor.matmul(ptw[:, 0:448], wrm[:, 0:128], wrm[:, 0:448],
                             start=True, stop=True)
        for buf, S in ((y1a, Y1_S), (y1b, Y1_S), (y2a, Y2_S), (y2b, Y2_S)):
            nc.gpsimd.memset(buf[:, 0:4, :], 0.0)
            nc.gpsimd.memset(buf[:, S - 8:S, :], 0.0)
            nc.gpsimd.memset(buf[:, 4:S - 8, 0:4], 0.0)
            nc.gpsimd.memset(buf[:, 4:S - 8, S - 8:S], 0.0)
        nc.gpsimd.memset(acc, 0.0)

        bufsA = [xo, y1a, y2a, None]
        bufsB = [xo, y1b, y2b, None]
        SH = 2  # shifted-half row offset of y buffers (= conv2/3 dilation)

        for pi, (pa, pb) in enumerate(PAIR_SEQ):
            parity = pa % 2
            if pi == 0:
                wt = wt0
            else:
                wt = wpool.tile([128, W_BLOB_COLS], BF16, tag="wt")
                # conv1 columns first -- they gate the pair's first matmuls
                nc.sync.dma_start(out=wt[:, 0:C2_COL], in_=w_in[pi, :, 0:C2_COL])
            nc.sync.dma_start(out=wt[:, C2_COL:W_BLOB_COLS],
                              in_=w_in[pi, :, C2_COL:W_BLOB_COLS])

            for l in range(3):
                g = GEOM[(l, parity)]
                d, off, oh, ow, woff, rch = (g["d"], g["off"], g["oh"], g["ow"],
                                             g["woff"], g["rch"])
                h0 = 0
                pend_a = None   # start row of a pending shift-DMA group
                while h0 < oh:
                    rr = min(rch, oh - h0)
                    n = rr * ow
                    pt = psum.tile([128, 512], F32, tag="pt")
                    for j in range(6):
                        kw = j % 3
                        kh0 = 0 if j < 3 else 2
                        rbase = h0 + kh0 * d + off
                        cbase = kw * d + off
                        if l == 0:
                            # conv1: both passes in one dense K=128 matmul
                            rhs = xo[0:128, rbase:rbase + rr, cbase:cbase + ow]
                            nc.tensor.matmul(pt[:, 0:n],
                                             wt[:, j * 128:(j + 1) * 128], rhs,
                                             start=(j == 0), stop=(j == 5))
                        else:
                            # conv2/3: per-pass K=128 kh-packed matmuls as two
                            # concurrent column tiles (0,0)/(0,64)
                            base_col = C2_COL if l == 1 else C3_COL
                            srcA, srcB = bufsA[l], bufsB[l]
                            rhsA = srcA[0:128, rbase:rbase + rr, cbase:cbase + ow]
                            rhsB = srcB[0:128, rbase:rbase + rr, cbase:cbase + ow]
                            cA = base_col + (2 * j) * 64
                            cB = base_col + (2 * j + 1) * 64
                            nc.tensor.matmul(pt[0:64, 0:n],
                                             wt[:, cA:cA + 64], rhsA,
                                             start=(j == 0), stop=(j == 5))
                            nc.tensor.matmul(pt[64:128, 0:n],
                                             wt[:, cB:cB + 64], rhsB,
                                             start=(j == 0), stop=(j == 5))
                    if l < 2:
                        dstA, dstB = bufsA[l + 1], bufsB[l + 1]
                        a = h0 + woff
                        # pass A plain half (partitions 0:64): ScalarE
                        nc.scalar.activation(
                            out=dstA[0:64, a:a + rr, woff:woff + ow],
                            in_=pt[0:64, 0:n].rearrange("p (r c) -> p r c", r=rr),
                            func=mybir.ActivationFunctionType.Relu,
                            bias=bit[0:64, l:l + 1])
                        # pass B plain half (partitions 64:128): DVE
                        nc.vector.tensor_scalar(
                            dstB[64:128, a:a + rr, woff:woff + ow],
                            pt[64:128, 0:n].rearrange("p (r c) -> p r c", r=rr),
                            bit[64:128, l:l + 1], 0.0,
                            mybir.AluOpType.add, mybir.AluOpType.max)
                        # shifted halves: rows [u0, u1) <- plain rows +SH,
                        # maintained with SBUF->SBUF DMAs (2 chunks per DMA)
                        # issued from the otherwise-idle sync/gpsimd queues
                        if pend_a is None:
                            pend_a = a
                        if h0 + rr >= oh or a + rr - pend_a >= 2 * rch:
                            u0, u1 = max(0, pend_a - SH), a + rr - SH
                            if u1 > u0:
                                nc.sync.dma_start(
                                    out=dstA[64:128, u0:u1, :],
                                    in_=dstA[0:64, u0 + SH:u1 + SH, :])
                                nc.gpsimd.dma_start(
                                    out=dstB[0:64, u0:u1, :],
                                    in_=dstB[64:128, u0 + SH:u1 + SH, :])
                            pend_a = None
                    else:
                        # conv3: bias then running channel max; acc is
                        # pair-packed [128, H*W] -- cross-half max happens in
                        # the final channel reduction
                        tmp = evpool.tile([128, 512], F32, tag="ev")
                        nc.scalar.activation(
                            out=tmp[:, 0:n], in_=pt[:, 0:n],
                            func=mybir.ActivationFunctionType.Identity,
                            bias=bit[:, 2:3])
                        nc.vector.tensor_max(
                            acc[:, h0 * W:h0 * W + n],
                            acc[:, h0 * W:h0 * W + n],
                            tmp[:, 0:n])
                    h0 += rr

        # channel-max reduction: PE-transpose 128-col blocks of acc into PSUM
        # ([128, 128] -> [128, 128]) and reduce over the free dim (both pair
        # halves' channels) on DVE.  72 blocks in 18 groups of 4; each
        # group's transposes start as soon as the conv3 maxes land.
        ident = bigpool.tile([128, 128], F32)
        masks.make_identity(nc, ident)
        red = bigpool.tile([128, 72], F32)
        NB = (H * W) // 128  # 72 blocks
        for g in range(NB // 4):
            ps = tpsum.tile([128, 512], F32, tag="tp")
            for b in range(4):
                blk = g * 4 + b
                nc.tensor.transpose(ps[:, b * 128:(b + 1) * 128],
                                    acc[:, blk * 128:(blk + 1) * 128],
                                    ident[:, :])
            nc.vector.tensor_reduce(out=red[:, g * 4:(g + 1) * 4],
                                    in_=ps.rearrange("p (b c) -> p b c", b=4),
                                    axis=mybir.AxisListType.X,
                                    op=mybir.AluOpType.max)
        ps2 = tpsum.tile([128, 512], F32, tag="tp")
        nc.tensor.transpose(ps2[0:72, 0:128], red[:, :], ident[:, :])
        rsb = bigpool.tile([72, 128], F32)
        nc.scalar.activation(out=rsb, in_=ps2[0:72, 0:128],
                             func=mybir.ActivationFunctionType.Sigmoid)
        nc.vector.tensor_scalar(rsb, rsb, 1e-4, 1.0 - 1e-4,
                                mybir.AluOpType.max, mybir.AluOpType.min)
        nc.sync.dma_start(
            out=o_out.ap().rearrange("a (c r) -> a c r", r=128), in_=rsb)
    nc.compile()
    return nc


def _fold_weights(perms, dcn_w, dcn_b, conv2_w, conv2_b, conv3_w, conv3_b,
                  bn_gamma, bn_beta, bn_mean, bn_var):
    """Fold rotations/permutation/BN on the host. Returns (w_blob, biases).

    BN scale is folded into the conv weights (per OUT channel); biases keep
    the BN shift + scaled conv bias.
    """
    biases = np.empty((128, 3), np.float32)
    scs = []
    conv_bs = [dcn_b, conv2_b, conv3_b]
    for l in range(3):
        s = bn_gamma[l] / np.sqrt(bn_var[l] + BN_EPS)
        scs.append(s)
        bl = bn_beta[l] - bn_mean[l] * s + conv_bs[l] * s
        biases[0:C, l] = bl
        biases[C:128, l] = bl

    w_blob = np.zeros((4, 128, W_BLOB_COLS), np.float32)
    base_ws = [dcn_w, conv2_w, conv3_w]
    for pi, pair in enumerate(PAIR_SEQ):
        for half, p in enumerate(pair):
            k = p % 4
            # conv1: rotation + channel permutation + BN scale folded
            w1 = np.rot90(base_ws[0], k=-k, axes=(-2, -1))
            w1 = np.einsum('omhw,mj->ojhw', w1, perms[p], optimize=True)
            w1 = w1 * scs[0][:, None, None, None]
            for j in range(6):
                kw = j % 3
                col = j * 128 + half * 64
                if j < 3:
                    w_blob[pi, 0:C, col:col + C] = w1[:, :, 0, kw].T
                    w_blob[pi, C:128, col:col + C] = w1[:, :, 1, kw].T
                else:
                    w_blob[pi, 0:C, col:col + C] = w1[:, :, 2, kw].T
            # conv2/conv3: kh-packed K=128 per-pass tiles.  Pass A buffers
            # are [plain | +2shift] (plain rows 0:64 -> kh0/kh2 taps on K
            # rows 0:64); pass B buffers are flipped [+2shift | plain]
            # (plain on K rows 64:128).
            for l, base_col in ((1, C2_COL), (2, C3_COL)):
                wl = np.rot90(base_ws[l], k=-k, axes=(-2, -1))
                wl = wl * scs[l][:, None, None, None]
                for j in range(6):
                    kw = j % 3
                    col = base_col + (2 * j + half) * 64
                    if half == 0:   # plain on rows 0:64, shifted on 64:128
                        if j < 3:
                            w_blob[pi, 0:C, col:col + C] = wl[:, :, 0, kw].T
                            w_blob[pi, C:128, col:col + C] = wl[:, :, 1, kw].T
                        else:
                            w_blob[pi, 0:C, col:col + C] = wl[:, :, 2, kw].T
                    else:           # flipped: shifted on rows 0:64, plain on 64:128
                        if j < 3:
                            w_blob[pi, C:128, col:col + C] = wl[:, :, 0, kw].T
                            w_blob[pi, 0:C, col:col + C] = wl[:, :, 1, kw].T
                        else:
                            w_blob[pi, C:128, col:col + C] = wl[:, :, 2, kw].T
    return w_blob, biases


def kernel(x, perms, dcn_w, dcn_b, conv2_w, conv2_b, conv3_w, conv3_b,
           bn_gamma, bn_beta, bn_mean, bn_var):
    global LAST_EXEC_NS
    x = np.ascontiguousarray(np.asarray(x, np.float32))
    args = [np.asarray(a, np.float32) for a in
            (perms, dcn_w, dcn_b, conv2_w, conv2_b, conv3_w, conv3_b,
             bn_gamma, bn_beta, bn_mean, bn_var)]
    w_blob, biases = _fold_weights(*args)
    w_blob = w_blob.astype(NP_BF16)
    x_bf = x.astype(NP_BF16)
    # pre-padded pair-packed image: [x + zero ring | same shifted up 1 row]
    P1 = PAD + 1
    x2 = np.zeros((B, 128, XO_S, XO_S), dtype=NP_BF16)
    x2[:, 0:C, P1:P1 + H, P1:P1 + W] = x_bf
    x2[:, C:128, P1 - 1:P1 - 1 + H, P1:P1 + W] = x_bf

    if "prog" not in _PROGRAM_CACHE:
        _PROGRAM_CACHE["prog"] = _build_program()
    nc = _PROGRAM_CACHE["prog"]

    in_maps = [{
        "x2_in": np.ascontiguousarray(x2[b]),
        "w_in": w_blob,
        "bi_in": biases,
    } for b in range(B)]

    r = run_bass_kernel_spmd(nc, in_maps, core_ids=list(range(B)), trace=TRACE)
    LAST_EXEC_NS = r.exec_time_ns
    out = np.stack([r.results[b]["o_out"].reshape(1, H, W) for b in range(B)])
    return out.astype(np.float32)
